# revision 71
# baseline (speedup 1.0000x reference)
"""DiT attention kernel for Trainium2 (Bass/Tile), data-parallel over batch.

Problem: B=8, S=1024, D=1024, H=16 heads, head_dim=64, fp32.
  q = x@wq.T; k = x@wk.T; v = x@wv.T          (per batch)
  attn = softmax(q k^T / sqrt(hd)); out = (attn v) @ wo.T

Sharding: batch is split 1:1 onto the 8 NeuronCores (pure data parallel,
no collectives). Weights are broadcast. Host pre-transposes x (per batch)
and the four weights, and converts them to fp16 (11-bit mantissa keeps
the end-to-end error ~1e-3, well inside the 2e-2 gate) which halves HBM
traffic and lets every matmul run 1 cycle/row at any moving length.

Cost-model-aware dataflow (matmul time = moving-dim rows only; K and M
are free):
  xT    [d, s] fp16 : DMA (host-transposed input)
  Q^T/K^T [o, s]    : fp16 matmuls, drained to f32r (N=512 moving)
  V_aug [s, h, 65] fp16: V projection with an appended ones column so the
                    flipped attnV matmul also produces softmax
                    denominators.
  per head h:     S^T[kpos, qpos] = K_h^T chunkT @ Q_h^T (K=64, N=512,
                  f32r), exp on ACT (scale=1/8 folded; no max-subtraction:
                  scores ~N(0,1)) -> et fp16 [kpos, 1024].
                  attnV FLIPPED: stationary = et[:, qc*128:...] (M=128
                  qpos), moving = V_aug[kpos, 65] fp16 -> psO2 accumulated
                  over kpos chunks. 65-row moving beats the 1024-row
                  moving of the natural orientation 2x.
  normalize:      reciprocal_approx_fast on the summed ones column, then
                  one fused DVE multiply-drain per qc group into raw2pair
                  [qpos, qc, hh, 64] fp16. No selector matmuls, no
                  partition-shift DMAs.
  transpose:      one SBUF->SBUF DMA-transpose instruction per head pair
                  turns raw2pair [s, 128d] into rawT [128d, sc, 128s]
                  (the 3D output form transposes each 128x128 block), at
                  ~0.9us on otherwise-idle DMA hardware - no PE cost.
  Y[s, o]         : lhsT = rawT chunk fp16, rhs = woT fp16 row tiles,
                  group-major so stores overlap the remaining matmuls; the
                  final two banks run sc-at-a-time into separate tiles so
                  the kernel tail is one short copy+DMA chain.

PSUM accumulation-group rule (measured on HW, bass_interp models it
per-cell and misses it): a start=True matmul wipes its WHOLE PSUM bank,
so at most one open accumulation group may live per bank. The flipped
attnV therefore runs one full qc-group (8 contiguous matmuls over all
kpos chunks of the PREVIOUS head) per kc slot, ping-ponging between the
two banks of its 4KB slot, software-pipelined one full head behind the
scores/exp stream; the normalize-drain lags two slots so a bank is fully
read before the next start wipes it.

Scheduling: Q/K projections for chunk oc+1 interleave into chunk oc's
heads as PE filler (the head alone is ACT-rate-bound by exp), emitted
sh-major with split half-drains so the next head's scores only wait on a
0.6us half-copy; two filler pieces land in each of the first two kc
slots to cover the psS-slot recycle against the previous head's last
exp. Weight DMAs ride the SP HWDGE queue behind the x stream (the ACT
queue must stay clear of DMA-seq configs or the exp stream stalls at the
sequencer). PSUM is exactly 16KB: psS 2x4KB (score/exp pipeline), psO2
1x4KB, filler 1x4KB.

PE work: 65536 (V) + 131072 (QK) + 131072 (scores) + 66560 (attnV) +
65536 (out) = 459776 rows ~= 192us at 2.4GHz, vs 532480 rows for the
natural orientation; cost-model wall 210.2us (91% PE busy). ACT (exp)
~134us, DVE ~57us fit underneath.
"""
import numpy as np
from contextlib import ExitStack

import concourse.bass as bass
import concourse.mybir as mybir
import concourse.tile as tile
from concourse import bacc
import concourse.bass_utils as bass_utils
from concourse.bass import ds

B, S, D, H = 8, 1024, 1024, 16
HD = D // H          # 64
P = 128
NCORES = 8
DC = D // P          # 8 chunks of the feature dim
SC = S // P          # 8 chunks of the sequence dim
NH = 512             # matmul moving-dim chunk (one PSUM bank)

f32 = mybir.dt.float32
f32r = mybir.dt.float32r
f16 = mybir.dt.float16
AF = mybir.ActivationFunctionType
ALU = mybir.AluOpType


def emit(tc, xT_d, wqT_d, wkT_d, wvT_d, woT_d, y_d):
    nc = tc.nc
    with ExitStack() as ctx:
        xp = ctx.enter_context(tc.tile_pool(name="xp", bufs=1))
        qkp = ctx.enter_context(tc.tile_pool(name="qkp", bufs=1))
        vp = ctx.enter_context(tc.tile_pool(name="vp", bufs=1))
        ep = ctx.enter_context(tc.tile_pool(name="ep", bufs=18))
        r2p = ctx.enter_context(tc.tile_pool(name="r2p", bufs=2))
        rtp = ctx.enter_context(tc.tile_pool(name="rtp", bufs=1))
        rcpp = ctx.enter_context(tc.tile_pool(name="rcpp", bufs=2))
        wp = ctx.enter_context(tc.tile_pool(name="wp", bufs=4))
        wvp = ctx.enter_context(tc.tile_pool(name="wvp", bufs=1))
        wop = ctx.enter_context(tc.tile_pool(name="wop", bufs=1))
        yp = ctx.enter_context(tc.tile_pool(name="yp", bufs=2))
        pp = ctx.enter_context(tc.tile_pool(name="pp", bufs=1, space="PSUM"))

        # PSUM slots: "ps" x2 (score/exp pipeline), "po" x1 (attnV accum),
        # "pq" x1 (Q/K filler). 4x4KB = 16KB exactly. The V projection and
        # output projection borrow all four as plain [P, 2NH] accumulators.
        def psum4(stem):
            return [
                pp.tile([P, 2 * NH], f32, tag="ps", bufs=2, name=f"{stem}0"),
                pp.tile([P, 2 * NH], f32, tag="ps", bufs=2, name=f"{stem}1"),
                pp.tile([P, 2 * NH], f32, tag="po", bufs=1, name=f"{stem}2"),
                pp.tile([P, 2 * NH], f32, tag="pq", bufs=1, name=f"{stem}3"),
            ]

        wqk_tiles = {}

        def load_wqk(oc, key, wd):
            if (key, oc) in wqk_tiles or oc >= DC:
                return
            wt = wp.tile([P, DC, P], f16, tag="wqk", name=f"w{key}{oc}")
            # wq/wk are host-blocked to [oc, p, dc, o]: this load is one DMA
            # of 128 contiguous 2KB descriptors. SP queue: the ACT queue
            # must stay clear of DMA-seq configs or the exp stream and the
            # scalar-engine drains stall behind them at the sequencer.
            nc.sync.dma_start(wt[:], wd[oc])
            wqk_tiles[(key, oc)] = wt

        wots = {}

        def load_wo(oh, dc):
            t = wop.tile([P, NH], f16, tag=f"wo{oh}_{dc}", bufs=1,
                         name=f"wo{oh}_{dc}")
            nc.sync.dma_start(t[:], woT_d[ds(dc * P, P), ds(oh * NH, NH)])
            wots[(oh, dc)] = t

        # ---- V projection: V_aug [s_part, sc, head, 65] fp16 ----
        # xT tiles (SP queue) and wv tiles (ACT queue) are loaded just in
        # time inside the first pass. The oh=1 wv halves live in separate
        # tiles loaded later, keeping 1MB out of the startup DMA window
        # (whole-tile deps only: split-tile half-loads raced on HW).
        V = vp.tile([P, SC, H, HD + 1], f16, tag="v")
        ones_t = yp.tile([P, H], f16, tag="ones", bufs=1)
        nc.vector.memset(ones_t[:], 1.0)
        for sc in range(SC):
            nc.vector.tensor_copy(V[:, sc, :, HD], ones_t[:])

        xts = []
        wvts = []
        wvbs = []

        def load_wvb(dc):
            t = wvp.tile([P, NH], f16, tag=f"wvb{dc}", name=f"wvb{dc}")
            nc.scalar.dma_start(t[:], wvT_d[ds(dc * P, P), NH:S])
            wvbs.append(t)

        def emit_v_pass(oh):
            psVs = psum4(f"psV{oh}_")

            def drain(sc):
                src = psVs[sc // 2][:, ds((sc % 2) * NH, NH)]
                dst = V[:, sc, ds(oh * 8, 8), 0:HD]
                if sc % 2 == 0:
                    nc.vector.tensor_copy(dst, src.rearrange("p (h e) -> p h e", e=HD))
                else:
                    nc.scalar.copy(dst, src.rearrange("p (h e) -> p h e", e=HD))

            for dc in range(DC):
                if oh == 0:
                    wvt = wvp.tile([P, 2 * NH], f16, tag=f"wv{dc}",
                                   name=f"wv{dc}")
                    if dc == 0:
                        # halves: the first V matmul only waits 128KB
                        nc.scalar.dma_start(wvt[:, 0:NH], wvT_d[0:P, 0:NH])
                        nc.scalar.dma_start(wvt[:, NH:S], wvT_d[0:P, NH:S])
                    else:
                        nc.scalar.dma_start(wvt[:], wvT_d[ds(dc * P, P), :])
                    wvts.append(wvt)
                    t = xp.tile([P, S], f16, tag=f"x{dc}")
                    if dc == 0:
                        # three pieces: the first V matmul only waits 32KB
                        nc.sync.dma_start(t[:, 0:P], xT_d[0:P, 0:P])
                        nc.sync.dma_start(t[:, P:NH], xT_d[0:P, P:NH])
                        nc.sync.dma_start(t[:, NH:S], xT_d[0:P, NH:S])
                    else:
                        nc.sync.dma_start(t[:], xT_d[ds(dc * P, P), :])
                    xts.append(t)
                for sc in range(SC):
                    nc.tensor.matmul(
                        psVs[sc // 2][:, ds((sc % 2) * NH, NH)],
                        xts[dc][:, ds(sc * P, P)],
                        wvts[dc][:, ds(oh * NH, NH)],
                        start=(dc == 0), stop=(dc == DC - 1))
                    if dc == DC - 1 and sc % 2 == 1:
                        # group complete: drain immediately so the next
                        # phase's PSUM slots free as early as possible
                        drain(sc - 1)
                        drain(sc)

        emit_v_pass(0)
        for dc in range(3, DC):
            load_wvb(dc)
        for occ in (0, 1):
            load_wqk(occ, "q", wqT_d)
            load_wqk(occ, "k", wkT_d)

        emit_v_pass(1)

        # ---- software-pipelined Q/K projection + attention ----
        QT, KT = {}, {}

        def qk_gen(oc, key, store, tag="pq"):
            """Generator: emits the oc-chunk Q/K projection in 8 pieces so it
            can be interleaved into an attention head's kc loop as PE filler
            (the head alone is ACT-rate-limited by exp). sh-major with split
            half-drains: the first half of QT/KT is in SBUF 4 pieces early,
            so the next head's scores never wait on a full [P, S] copy."""
            wt = wqk_tiles.pop((key, oc))
            ps = pp.tile([P, 2 * NH], f32, tag=tag,
                         bufs=1 if tag == "pq" else 2, name=f"ps{key}{oc}")
            dst = qkp.tile([P, S], f32r, tag=f"{key}{oc % 2}", name=f"t{key}{oc}")
            for sh in range(2):
                for dp in range(4):
                    if sh == 1 and dp == 0:
                        nc.vector.tensor_copy(dst[:, 0:NH], ps[:, 0:NH])
                    for dc in (2 * dp, 2 * dp + 1):
                        nc.tensor.matmul(
                            ps[:, ds(sh * NH, NH)], wt[:, dc, :],
                            xts[dc][:, ds(sh * NH, NH)],
                            start=(dc == 0), stop=(dc == DC - 1))
                    if sh == 0 or dp < 3:
                        yield
            nc.vector.tensor_copy(dst[:, NH:S], ps[:, NH:S])
            store[oc] = dst

        # Attention heads. PSUM accumulation-group rule (measured on HW): a
        # start=True matmul wipes its whole PSUM bank, so at most one open
        # accumulation group may live per bank. The flipped attnV therefore
        # runs one full qc-group (8 contiguous matmuls over all kc) per
        # slot, ping-ponging between the two banks of the "po" slot, and is
        # software-pipelined one FULL HEAD behind the scores/exp stream (so
        # every et tile of the previous head already exists). The
        # reciprocal+multiply normalize-drain lags two slots so the bank is
        # fully read before its next start wipes it.
        def attn_q(prev, qc):
            psO2, pe, ph = prev["psO2"], prev["ets"], prev["h"]
            for kc2 in range(SC):
                nc.tensor.matmul(
                    psO2[:, qc % 2, 0:HD + 1],
                    pe[kc2][:, ds(qc * P, P)], V[:, kc2, ph, :],
                    start=(kc2 == 0), stop=(kc2 == SC - 1))

        def drain_q(prev, qc):
            psO2 = prev["psO2"]
            rcp1 = rcpp.tile([P, 1], f32, tag="rc", bufs=4,
                             name=f"rcp{prev['h']}_{qc}")
            nc.vector.reciprocal_approx_fast(
                out=rcp1[:, 0:1], in_=psO2[:, qc % 2, HD:HD + 1])
            nc.vector.tensor_tensor(
                prev["raw2"][:, qc, prev["h"] % 2, :],
                psO2[:, qc % 2, 0:HD],
                rcp1.to_broadcast((P, HD)), ALU.mult)

        def emit_head(t, raw2pair, filler=None, prev=None):
            oc, hh = divmod(t, 2)
            ets = []
            if prev is not None:
                prev["psO2"] = pp.tile([P, 2, NH], f32, tag="po", bufs=1,
                                       name=f"psO{prev['h']}")
            for kc in range(SC):
                psS = pp.tile([P, 2 * NH], f32, tag="ps", bufs=2,
                              name=f"psS{t}_{kc}")
                lhsT = KT[oc][ds(hh * HD, HD), ds(kc * P, P)]
                for qh in range(2):
                    nc.tensor.matmul(
                        psS[:, ds(qh * NH, NH)], lhsT,
                        QT[oc][ds(hh * HD, HD), ds(qh * NH, NH)],
                        start=True, stop=True)
                et = ep.tile([P, S], f16, tag="e", name=f"et{t}_{kc}")
                nc.scalar.activation(et[:], psS[:], AF.Exp, scale=0.125)
                ets.append(et)
                if prev is not None:
                    if kc >= 2:
                        drain_q(prev, kc - 2)
                    attn_q(prev, kc)
                if filler is not None:
                    next(filler, None)
                    if kc < 2:
                        next(filler, None)
            if prev is not None:
                drain_q(prev, SC - 2)
                drain_q(prev, SC - 1)
            if filler is not None:
                for _ in filler:
                    pass
            return {"h": t, "ets": ets, "raw2": raw2pair}

        def emit_attn_tail(prev, filler=None):
            prev["psO2"] = pp.tile([P, 2, NH], f32, tag="po", bufs=1,
                                   name=f"psO{prev['h']}")
            for qc in range(SC):
                if qc >= 2:
                    drain_q(prev, qc - 2)
                attn_q(prev, qc)
                if filler is not None:
                    next(filler, None)
            drain_q(prev, SC - 2)
            drain_q(prev, SC - 1)

        def emit_transpose(oc):
            # one DMA-transpose per pair: [s, 128d] -> [128d, sc, 128s]
            # (3D output form = per-128x128-block transpose), off the PE
            rt = rtp.tile([P, SC, P], f16, tag=f"rt{oc}", name=f"rawT{oc}")
            nc.sync.dma_start_transpose(rt[:], raw2s[oc][:])
            rawT[oc] = rt

        rawT, raw2s = {}, {}
        # QK0: Q on a "ps" slot, K on the "pq" slot, interleaved so both
        # accumulate concurrently and drain back to back
        g_q0 = qk_gen(0, "q", QT, tag="ps")
        g_k0 = qk_gen(0, "k", KT, tag="pq")
        for _ in range(12):
            next(g_q0, None)
            next(g_k0, None)
        load_wqk(1, "q", wqT_d)
        load_wqk(1, "k", wkT_d)
        def store_y(oh, sc, psY, engine_alt):
            # reuse the (long dead) xT slots as output staging
            yt = xp.tile([P, NH], f32, tag=f"x{sc}", name=f"yt{oh}_{sc}")
            src_ap = psY[:, ds((sc % 2) * NH, NH)]
            if engine_alt:
                nc.vector.tensor_copy(yt[:], src_ap)
            else:
                nc.scalar.copy(yt[:], src_ap)
            nc.sync.dma_start(y_d[ds(sc * P, P), ds(oh * NH, NH)], yt[:])

        # out-proj prework: heads 14/15 have no Q/K filler left, so the
        # sc=6,7 (oh=0) output-projection group accumulates its dc<7 steps
        # there instead, in the freed "pq" slot; dc=7 + the drain happen
        # after the attnV tail
        owork = {}

        def owork_gen(dcs):
            for dc in dcs:
                for sc in (6, 7):
                    nc.tensor.matmul(
                        owork["psY"][:, ds((sc % 2) * NH, NH)],
                        rawT[dc][:, sc, :], wots[(0, dc)],
                        start=(dc == 0), stop=False)
                yield

        prev = None
        for t in range(2 * DC):
            oc, hh = divmod(t, 2)
            if hh == 0:
                load_wqk(oc + 2, "q", wqT_d)
                load_wqk(oc + 2, "k", wkT_d)
                for i in (2 * oc, 2 * oc + 1):
                    load_wo(i // DC, i % DC)
                raw2s[oc] = r2p.tile([P, SC, 2, HD], f16, tag="r2",
                                     name=f"raw2_{oc}")
            filler = (qk_gen(oc + 1, "qk"[hh], QT if hh == 0 else KT)
                      if oc + 1 < DC else None)
            prev = emit_head(t, raw2s[oc], filler=filler, prev=prev)
            if hh == 0 and oc >= 1:
                # pair oc-1's raw2 completed during this head's slots
                emit_transpose(oc - 1)
        emit_attn_tail(prev)
        emit_transpose(DC - 1)

        # ---- output projection Y[s, o], sc-major ----
        # each [P, NH] PSUM bank finishes its own dc loop, then drains and
        # stores while the next bank's matmuls run, so the kernel tail is
        # just one copy+DMA chain
        for oh in range(2):
            psYs = psum4(f"psY{oh}_")
            grange = (0, 1, 2, 3) if oh == 0 else (0, 1, 2)
            for g in grange:
                for dc in range(DC):
                    wot = wots[(oh, dc)]
                    for sc in (2 * g, 2 * g + 1):
                        nc.tensor.matmul(
                            psYs[g][:, ds((sc % 2) * NH, NH)],
                            rawT[dc][:, sc, :], wot[:],
                            start=(dc == 0), stop=(dc == DC - 1))
                for sc in (2 * g, 2 * g + 1):
                    store_y(oh, sc, psYs[g], sc % 2 == 0)
            if oh == 1:
                # final two banks run sc-at-a-time so the very last
                # copy+DMA chain hides behind sc=7's matmuls; sc=7 stores
                # in four column-quarters (engines alternating) so the
                # kernel tail is one tiny copy+DMA chain
                psY7 = pp.tile([P, NH], f32, tag="po", bufs=1, name="psY7")
                for sc in (6, 7):
                    # separate tiles: sc6's store must not WAR-serialize
                    # against sc7's start matmul
                    psY = psYs[3] if sc == 6 else psY7
                    off = 0 if sc == 7 else 0
                    for dc in range(DC):
                        nc.tensor.matmul(
                            psY[:, ds(off, NH)],
                            rawT[dc][:, sc, :], wots[(oh, dc)],
                            start=(dc == 0), stop=(dc == DC - 1))
                    yt = xp.tile([P, NH], f32, tag=f"x{sc}", name=f"yt1_{sc}")
                    if sc == 6:
                        nc.vector.tensor_copy(yt[:], psY[:, 0:NH])
                    else:
                        nc.scalar.copy(yt[:], psY[:, 0:NH])
                    nc.sync.dma_start(y_d[ds(sc * P, P), ds(oh * NH, NH)],
                                      yt[:])


def build_nc():
    nc = bacc.Bacc("TRN2", target_bir_lowering=False, debug=False,
                   enable_asserts=False, num_devices=NCORES)
    xT_d = nc.dram_tensor("xT", (D, S), f16, kind="ExternalInput").ap()
    wqT_d = nc.dram_tensor("wqT", (DC, P, DC, P), f16, kind="ExternalInput").ap()
    wkT_d = nc.dram_tensor("wkT", (DC, P, DC, P), f16, kind="ExternalInput").ap()
    wvT_d = nc.dram_tensor("wvT", (D, D), f16, kind="ExternalInput").ap()
    woT_d = nc.dram_tensor("woT", (D, D), f16, kind="ExternalInput").ap()
    y_d = nc.dram_tensor("y", (S, D), f32, kind="ExternalOutput").ap()
    with tile.TileContext(nc) as tc:
        emit(tc, xT_d, wqT_d, wkT_d, wvT_d, woT_d, y_d)
    nc.compile()
    return nc


_NC_CACHE = None


def _get_nc():
    global _NC_CACHE
    if _NC_CACHE is None:
        _NC_CACHE = build_nc()
    return _NC_CACHE


def _block_qk(w):
    # wT[dc*P+p, oc*P+o] -> [oc, p, dc, o] so each per-oc stationary load is
    # a single DMA of contiguous 2KB-per-partition descriptors
    wT = np.asarray(w, np.float32).T
    return np.ascontiguousarray(
        wT.reshape(DC, P, DC, P).transpose(2, 1, 0, 3)).astype(np.float16)


def make_in_maps(x, wq, wk, wv, wo):
    x = np.asarray(x, dtype=np.float32)
    wqT = _block_qk(wq)
    wkT = _block_qk(wk)
    wvT = np.ascontiguousarray(np.asarray(wv, np.float32).T).astype(np.float16)
    woT = np.ascontiguousarray(np.asarray(wo, np.float32).T).astype(np.float16)
    in_maps = []
    for b in range(B):
        in_maps.append({
            "xT": np.ascontiguousarray(x[b].T).astype(np.float16),
            "wqT": wqT, "wkT": wkT, "wvT": wvT, "woT": woT,
        })
    return in_maps


def kernel(x, wq, wk, wv, wo):
    nc = _get_nc()
    in_maps = make_in_maps(x, wq, wk, wv, wo)
    res = bass_utils.run_bass_kernel_spmd(nc, in_maps, core_ids=list(range(NCORES)))
    return np.stack([res.results[b]["y"] for b in range(B)], axis=0)


# revision 72
# speedup vs baseline: 1.0006x; 1.0006x over previous
"""DiT attention kernel for Trainium2 (Bass/Tile), data-parallel over batch.

Problem: B=8, S=1024, D=1024, H=16 heads, head_dim=64, fp32.
  q = x@wq.T; k = x@wk.T; v = x@wv.T          (per batch)
  attn = softmax(q k^T / sqrt(hd)); out = (attn v) @ wo.T

Sharding: batch is split 1:1 onto the 8 NeuronCores (pure data parallel,
no collectives). Weights are broadcast. Host pre-transposes x (per batch)
and the four weights, and converts them to fp16 (11-bit mantissa keeps
the end-to-end error ~1e-3, well inside the 2e-2 gate) which halves HBM
traffic and lets every matmul run 1 cycle/row at any moving length.

Cost-model-aware dataflow (matmul time = moving-dim rows only; K and M
are free):
  xT    [d, s] fp16 : DMA (host-transposed input)
  Q^T/K^T [o, s]    : fp16 matmuls, drained to f32r (N=512 moving)
  V_aug [s, h, 65] fp16: V projection with an appended ones column so the
                    flipped attnV matmul also produces softmax
                    denominators.
  per head h:     S^T[kpos, qpos] = K_h^T chunkT @ Q_h^T (K=64, N=512,
                  f32r), exp on ACT (scale=1/8 folded; no max-subtraction:
                  scores ~N(0,1)) -> et fp16 [kpos, 1024].
                  attnV FLIPPED: stationary = et[:, qc*128:...] (M=128
                  qpos), moving = V_aug[kpos, 65] fp16 -> psO2 accumulated
                  over kpos chunks. 65-row moving beats the 1024-row
                  moving of the natural orientation 2x.
  normalize:      reciprocal_approx_fast on the summed ones column, then
                  one fused DVE multiply-drain per qc group into raw2pair
                  [qpos, qc, hh, 64] fp16. No selector matmuls, no
                  partition-shift DMAs.
  transpose:      one SBUF->SBUF DMA-transpose instruction per head pair
                  turns raw2pair [s, 128d] into rawT [128d, sc, 128s]
                  (the 3D output form transposes each 128x128 block), at
                  ~0.9us on otherwise-idle DMA hardware - no PE cost.
  Y[s, o]         : lhsT = rawT chunk fp16, rhs = woT fp16 row tiles,
                  group-major so stores overlap the remaining matmuls; the
                  final two banks run sc-at-a-time into separate tiles so
                  the kernel tail is one short copy+DMA chain.

PSUM accumulation-group rule (measured on HW, bass_interp models it
per-cell and misses it): a start=True matmul wipes its WHOLE PSUM bank,
so at most one open accumulation group may live per bank. The flipped
attnV therefore runs one full qc-group (8 contiguous matmuls over all
kpos chunks of the PREVIOUS head) per kc slot, ping-ponging between the
two banks of its 4KB slot, software-pipelined one full head behind the
scores/exp stream; the normalize-drain lags two slots so a bank is fully
read before the next start wipes it.

Scheduling: Q/K projections for chunk oc+1 interleave into chunk oc's
heads as PE filler (the head alone is ACT-rate-bound by exp), emitted
sh-major with split half-drains so the next head's scores only wait on a
0.6us half-copy; two filler pieces land in each of the first two kc
slots to cover the psS-slot recycle against the previous head's last
exp. Weight DMAs ride the SP HWDGE queue behind the x stream (the ACT
queue must stay clear of DMA-seq configs or the exp stream stalls at the
sequencer). PSUM is exactly 16KB: psS 2x4KB (score/exp pipeline), psO2
1x4KB, filler 1x4KB.

PE work: 65536 (V) + 131072 (QK) + 131072 (scores) + 66560 (attnV) +
65536 (out) = 459776 rows ~= 192us at 2.4GHz, vs 532480 rows for the
natural orientation; cost-model wall 210.2us (91% PE busy). ACT (exp)
~134us, DVE ~57us fit underneath.
"""
import numpy as np
from contextlib import ExitStack

import concourse.bass as bass
import concourse.mybir as mybir
import concourse.tile as tile
from concourse import bacc
import concourse.bass_utils as bass_utils
from concourse.bass import ds

B, S, D, H = 8, 1024, 1024, 16
HD = D // H          # 64
P = 128
NCORES = 8
DC = D // P          # 8 chunks of the feature dim
SC = S // P          # 8 chunks of the sequence dim
NH = 512             # matmul moving-dim chunk (one PSUM bank)

f32 = mybir.dt.float32
f32r = mybir.dt.float32r
f16 = mybir.dt.float16
AF = mybir.ActivationFunctionType
ALU = mybir.AluOpType


def emit(tc, xT_d, wqT_d, wkT_d, wvT_d, woT_d, y_d):
    nc = tc.nc
    with ExitStack() as ctx:
        xp = ctx.enter_context(tc.tile_pool(name="xp", bufs=1))
        qkp = ctx.enter_context(tc.tile_pool(name="qkp", bufs=1))
        vp = ctx.enter_context(tc.tile_pool(name="vp", bufs=1))
        ep = ctx.enter_context(tc.tile_pool(name="ep", bufs=18))
        r2p = ctx.enter_context(tc.tile_pool(name="r2p", bufs=2))
        rtp = ctx.enter_context(tc.tile_pool(name="rtp", bufs=1))
        rcpp = ctx.enter_context(tc.tile_pool(name="rcpp", bufs=2))
        wp = ctx.enter_context(tc.tile_pool(name="wp", bufs=4))
        wvp = ctx.enter_context(tc.tile_pool(name="wvp", bufs=1))
        wop = ctx.enter_context(tc.tile_pool(name="wop", bufs=1))
        yp = ctx.enter_context(tc.tile_pool(name="yp", bufs=2))
        pp = ctx.enter_context(tc.tile_pool(name="pp", bufs=1, space="PSUM"))

        # PSUM slots: "ps" x2 (score/exp pipeline), "po" x1 (attnV accum),
        # "pq" x1 (Q/K filler). 4x4KB = 16KB exactly. The V projection and
        # output projection borrow all four as plain [P, 2NH] accumulators.
        def psum4(stem):
            return [
                pp.tile([P, 2 * NH], f32, tag="ps", bufs=2, name=f"{stem}0"),
                pp.tile([P, 2 * NH], f32, tag="ps", bufs=2, name=f"{stem}1"),
                pp.tile([P, 2 * NH], f32, tag="po", bufs=1, name=f"{stem}2"),
                pp.tile([P, 2 * NH], f32, tag="pq", bufs=1, name=f"{stem}3"),
            ]

        wqk_tiles = {}

        def load_wqk(oc, key, wd):
            if (key, oc) in wqk_tiles or oc >= DC:
                return
            wt = wp.tile([P, DC, P], f16, tag="wqk", name=f"w{key}{oc}")
            # wq/wk are host-blocked to [oc, p, dc, o]: this load is one DMA
            # of 128 contiguous 2KB descriptors. SP queue: the ACT queue
            # must stay clear of DMA-seq configs or the exp stream and the
            # scalar-engine drains stall behind them at the sequencer.
            nc.sync.dma_start(wt[:], wd[oc])
            wqk_tiles[(key, oc)] = wt

        wots = {}

        def load_wo(oh, dc):
            t = wop.tile([P, NH], f16, tag=f"wo{oh}_{dc}", bufs=1,
                         name=f"wo{oh}_{dc}")
            nc.sync.dma_start(t[:], woT_d[ds(dc * P, P), ds(oh * NH, NH)])
            wots[(oh, dc)] = t

        # ---- V projection: V_aug [s_part, sc, head, 65] fp16 ----
        # xT tiles (SP queue) and wv tiles (ACT queue) are loaded just in
        # time inside the first pass. The oh=1 wv halves live in separate
        # tiles loaded later, keeping 1MB out of the startup DMA window
        # (whole-tile deps only: split-tile half-loads raced on HW).
        V = vp.tile([P, SC, H, HD + 1], f16, tag="v")
        ones_t = yp.tile([P, H], f16, tag="ones", bufs=1)
        nc.vector.memset(ones_t[:], 1.0)
        for sc in range(SC):
            nc.vector.tensor_copy(V[:, sc, :, HD], ones_t[:])

        xts = []
        wvts = []
        wvbs = []

        def load_wvb(dc):
            t = wvp.tile([P, NH], f16, tag=f"wvb{dc}", name=f"wvb{dc}")
            nc.scalar.dma_start(t[:], wvT_d[ds(dc * P, P), NH:S])
            wvbs.append(t)

        def emit_v_pass(oh):
            psVs = psum4(f"psV{oh}_")

            def drain(sc):
                src = psVs[sc // 2][:, ds((sc % 2) * NH, NH)]
                dst = V[:, sc, ds(oh * 8, 8), 0:HD]
                if sc % 2 == 0:
                    nc.vector.tensor_copy(dst, src.rearrange("p (h e) -> p h e", e=HD))
                else:
                    nc.scalar.copy(dst, src.rearrange("p (h e) -> p h e", e=HD))

            for dc in range(DC):
                if oh == 0:
                    wvt = wvp.tile([P, 2 * NH], f16, tag=f"wv{dc}",
                                   name=f"wv{dc}")
                    if dc == 0:
                        # halves: the first V matmul only waits 128KB
                        nc.scalar.dma_start(wvt[:, 0:NH], wvT_d[0:P, 0:NH])
                        nc.scalar.dma_start(wvt[:, NH:S], wvT_d[0:P, NH:S])
                    else:
                        nc.scalar.dma_start(wvt[:], wvT_d[ds(dc * P, P), :])
                    wvts.append(wvt)
                    t = xp.tile([P, S], f16, tag=f"x{dc}")
                    if dc == 0:
                        # three pieces: the first V matmul only waits 32KB
                        nc.sync.dma_start(t[:, 0:P], xT_d[0:P, 0:P])
                        nc.sync.dma_start(t[:, P:NH], xT_d[0:P, P:NH])
                        nc.sync.dma_start(t[:, NH:S], xT_d[0:P, NH:S])
                    else:
                        nc.sync.dma_start(t[:], xT_d[ds(dc * P, P), :])
                    xts.append(t)
                for sc in range(SC):
                    nc.tensor.matmul(
                        psVs[sc // 2][:, ds((sc % 2) * NH, NH)],
                        xts[dc][:, ds(sc * P, P)],
                        wvts[dc][:, ds(oh * NH, NH)],
                        start=(dc == 0), stop=(dc == DC - 1))
                    if dc == DC - 1 and sc % 2 == 1:
                        # group complete: drain immediately so the next
                        # phase's PSUM slots free as early as possible
                        drain(sc - 1)
                        drain(sc)

        emit_v_pass(0)
        for dc in range(3, DC):
            load_wvb(dc)
        for occ in (0, 1):
            load_wqk(occ, "q", wqT_d)
            load_wqk(occ, "k", wkT_d)

        emit_v_pass(1)

        # ---- software-pipelined Q/K projection + attention ----
        QT, KT = {}, {}

        def qk_gen(oc, key, store, tag="pq"):
            """Generator: emits the oc-chunk Q/K projection in 8 pieces so it
            can be interleaved into an attention head's kc loop as PE filler
            (the head alone is ACT-rate-limited by exp). sh-major with split
            half-drains: the first half of QT/KT is in SBUF 4 pieces early,
            so the next head's scores never wait on a full [P, S] copy."""
            wt = wqk_tiles.pop((key, oc))
            ps = pp.tile([P, 2 * NH], f32, tag=tag,
                         bufs=1 if tag == "pq" else 2, name=f"ps{key}{oc}")
            dst = qkp.tile([P, S], f32r, tag=f"{key}{oc % 2}", name=f"t{key}{oc}")
            for sh in range(2):
                for dp in range(4):
                    if sh == 1 and dp == 0:
                        nc.vector.tensor_copy(dst[:, 0:NH], ps[:, 0:NH])
                    for dc in (2 * dp, 2 * dp + 1):
                        nc.tensor.matmul(
                            ps[:, ds(sh * NH, NH)], wt[:, dc, :],
                            xts[dc][:, ds(sh * NH, NH)],
                            start=(dc == 0), stop=(dc == DC - 1))
                    if sh == 0 or dp < 3:
                        yield
            nc.vector.tensor_copy(dst[:, NH:S], ps[:, NH:S])
            store[oc] = dst

        # Attention heads. PSUM accumulation-group rule (measured on HW): a
        # start=True matmul wipes its whole PSUM bank, so at most one open
        # accumulation group may live per bank. The flipped attnV therefore
        # runs one full qc-group (8 contiguous matmuls over all kc) per
        # slot, ping-ponging between the two banks of the "po" slot, and is
        # software-pipelined one FULL HEAD behind the scores/exp stream (so
        # every et tile of the previous head already exists). The
        # reciprocal+multiply normalize-drain lags two slots so the bank is
        # fully read before its next start wipes it.
        def attn_q(prev, qc):
            psO2, pe, ph = prev["psO2"], prev["ets"], prev["h"]
            for kc2 in range(SC):
                nc.tensor.matmul(
                    psO2[:, qc % 2, 0:HD + 1],
                    pe[kc2][:, ds(qc * P, P)], V[:, kc2, ph, :],
                    start=(kc2 == 0), stop=(kc2 == SC - 1))

        def drain_q(prev, qc):
            psO2 = prev["psO2"]
            rcp1 = rcpp.tile([P, 1], f32, tag="rc", bufs=4,
                             name=f"rcp{prev['h']}_{qc}")
            nc.vector.reciprocal_approx_fast(
                out=rcp1[:, 0:1], in_=psO2[:, qc % 2, HD:HD + 1])
            nc.vector.tensor_tensor(
                prev["raw2"][:, qc, prev["h"] % 2, :],
                psO2[:, qc % 2, 0:HD],
                rcp1.to_broadcast((P, HD)), ALU.mult)

        def emit_head(t, raw2pair, filler=None, prev=None):
            oc, hh = divmod(t, 2)
            ets = []
            if prev is not None:
                prev["psO2"] = pp.tile([P, 2, NH], f32, tag="po", bufs=1,
                                       name=f"psO{prev['h']}")
            for kc in range(SC):
                psS = pp.tile([P, 2 * NH], f32, tag="ps", bufs=2,
                              name=f"psS{t}_{kc}")
                lhsT = KT[oc][ds(hh * HD, HD), ds(kc * P, P)]
                for qh in range(2):
                    nc.tensor.matmul(
                        psS[:, ds(qh * NH, NH)], lhsT,
                        QT[oc][ds(hh * HD, HD), ds(qh * NH, NH)],
                        start=True, stop=True)
                et = ep.tile([P, S], f16, tag="e", name=f"et{t}_{kc}")
                nc.scalar.activation(et[:], psS[:], AF.Exp, scale=0.125)
                ets.append(et)
                if prev is not None:
                    if kc >= 2:
                        drain_q(prev, kc - 2)
                    attn_q(prev, kc)
                if filler is not None:
                    next(filler, None)
                    if kc < 2:
                        next(filler, None)
            if prev is not None:
                drain_q(prev, SC - 2)
                drain_q(prev, SC - 1)
            if filler is not None:
                for _ in filler:
                    pass
            return {"h": t, "ets": ets, "raw2": raw2pair}

        def emit_attn_tail(prev, filler=None):
            prev["psO2"] = pp.tile([P, 2, NH], f32, tag="po", bufs=1,
                                   name=f"psO{prev['h']}")
            for qc in range(SC):
                if qc >= 2:
                    drain_q(prev, qc - 2)
                attn_q(prev, qc)
                if filler is not None:
                    next(filler, None)
            drain_q(prev, SC - 2)
            drain_q(prev, SC - 1)

        def emit_transpose(oc):
            # one DMA-transpose per pair: [s, 128d] -> [128d, sc, 128s]
            # (3D output form = per-128x128-block transpose), off the PE
            rt = rtp.tile([P, SC, P], f16, tag=f"rt{oc}", name=f"rawT{oc}")
            nc.sync.dma_start_transpose(rt[:], raw2s[oc][:])
            rawT[oc] = rt

        rawT, raw2s = {}, {}
        # QK0: Q on a "ps" slot, K on the "pq" slot, interleaved so both
        # accumulate concurrently and drain back to back
        g_q0 = qk_gen(0, "q", QT, tag="ps")
        g_k0 = qk_gen(0, "k", KT, tag="pq")
        for _ in range(12):
            next(g_q0, None)
            next(g_k0, None)
        load_wqk(1, "q", wqT_d)
        load_wqk(1, "k", wkT_d)
        def store_y(oh, sc, psY, engine_alt):
            # reuse the (long dead) xT slots as output staging
            yt = xp.tile([P, NH], f32, tag=f"x{sc}", name=f"yt{oh}_{sc}")
            src_ap = psY[:, ds((sc % 2) * NH, NH)]
            if engine_alt:
                nc.vector.tensor_copy(yt[:], src_ap)
            else:
                nc.scalar.copy(yt[:], src_ap)
            nc.sync.dma_start(y_d[ds(sc * P, P), ds(oh * NH, NH)], yt[:])

        # out-proj prework: heads 14/15 have no Q/K filler left, so the
        # sc=6,7 (oh=0) output-projection group accumulates its dc<7 steps
        # there instead, in the freed "pq" slot; dc=7 + the drain happen
        # after the attnV tail
        owork = {}

        def owork_gen(dcs):
            for dc in dcs:
                for sc in (6, 7):
                    nc.tensor.matmul(
                        owork["psY"][:, ds((sc % 2) * NH, NH)],
                        rawT[dc][:, sc, :], wots[(0, dc)],
                        start=(dc == 0), stop=False)
                yield

        prev = None
        for t in range(2 * DC):
            oc, hh = divmod(t, 2)
            if hh == 0:
                load_wqk(oc + 2, "q", wqT_d)
                load_wqk(oc + 2, "k", wkT_d)
                for i in (2 * oc, 2 * oc + 1):
                    load_wo(i // DC, i % DC)
                raw2s[oc] = r2p.tile([P, SC, 2, HD], f16, tag="r2",
                                     name=f"raw2_{oc}")
            filler = (qk_gen(oc + 1, "qk"[hh], QT if hh == 0 else KT)
                      if oc + 1 < DC else None)
            prev = emit_head(t, raw2s[oc], filler=filler, prev=prev)
            if hh == 0 and oc >= 1:
                # pair oc-1's raw2 completed during this head's slots
                emit_transpose(oc - 1)
        emit_attn_tail(prev)
        emit_transpose(DC - 1)

        # ---- output projection Y[s, o], sc-major ----
        # each [P, NH] PSUM bank finishes its own dc loop, then drains and
        # stores while the next bank's matmuls run, so the kernel tail is
        # just one copy+DMA chain
        for oh in range(2):
            psYs = psum4(f"psY{oh}_")
            # pq/po slots free before the ps slots (which wait on head 15's
            # last exps), so their groups are emitted first
            grange = (3, 2, 0, 1) if oh == 0 else (2, 0, 1)
            for g in grange:
                for dc in range(DC):
                    wot = wots[(oh, dc)]
                    for sc in (2 * g, 2 * g + 1):
                        nc.tensor.matmul(
                            psYs[g][:, ds((sc % 2) * NH, NH)],
                            rawT[dc][:, sc, :], wot[:],
                            start=(dc == 0), stop=(dc == DC - 1))
                for sc in (2 * g, 2 * g + 1):
                    store_y(oh, sc, psYs[g], sc % 2 == 0)
            if oh == 1:
                # final two banks run sc-at-a-time so the very last
                # copy+DMA chain hides behind sc=7's matmuls; sc=7 stores
                # in four column-quarters (engines alternating) so the
                # kernel tail is one tiny copy+DMA chain
                psY7 = pp.tile([P, NH], f32, tag="po", bufs=1, name="psY7")
                for sc in (6, 7):
                    # separate tiles: sc6's store must not WAR-serialize
                    # against sc7's start matmul
                    psY = psYs[3] if sc == 6 else psY7
                    off = 0 if sc == 7 else 0
                    for dc in range(DC):
                        nc.tensor.matmul(
                            psY[:, ds(off, NH)],
                            rawT[dc][:, sc, :], wots[(oh, dc)],
                            start=(dc == 0), stop=(dc == DC - 1))
                    yt = xp.tile([P, NH], f32, tag=f"x{sc}", name=f"yt1_{sc}")
                    if sc == 6:
                        nc.vector.tensor_copy(yt[:], psY[:, 0:NH])
                    else:
                        nc.scalar.copy(yt[:], psY[:, 0:NH])
                    nc.sync.dma_start(y_d[ds(sc * P, P), ds(oh * NH, NH)],
                                      yt[:])


def build_nc():
    nc = bacc.Bacc("TRN2", target_bir_lowering=False, debug=False,
                   enable_asserts=False, num_devices=NCORES)
    xT_d = nc.dram_tensor("xT", (D, S), f16, kind="ExternalInput").ap()
    wqT_d = nc.dram_tensor("wqT", (DC, P, DC, P), f16, kind="ExternalInput").ap()
    wkT_d = nc.dram_tensor("wkT", (DC, P, DC, P), f16, kind="ExternalInput").ap()
    wvT_d = nc.dram_tensor("wvT", (D, D), f16, kind="ExternalInput").ap()
    woT_d = nc.dram_tensor("woT", (D, D), f16, kind="ExternalInput").ap()
    y_d = nc.dram_tensor("y", (S, D), f32, kind="ExternalOutput").ap()
    with tile.TileContext(nc) as tc:
        emit(tc, xT_d, wqT_d, wkT_d, wvT_d, woT_d, y_d)
    nc.compile()
    return nc


_NC_CACHE = None


def _get_nc():
    global _NC_CACHE
    if _NC_CACHE is None:
        _NC_CACHE = build_nc()
    return _NC_CACHE


def _block_qk(w):
    # wT[dc*P+p, oc*P+o] -> [oc, p, dc, o] so each per-oc stationary load is
    # a single DMA of contiguous 2KB-per-partition descriptors
    wT = np.asarray(w, np.float32).T
    return np.ascontiguousarray(
        wT.reshape(DC, P, DC, P).transpose(2, 1, 0, 3)).astype(np.float16)


def make_in_maps(x, wq, wk, wv, wo):
    x = np.asarray(x, dtype=np.float32)
    wqT = _block_qk(wq)
    wkT = _block_qk(wk)
    wvT = np.ascontiguousarray(np.asarray(wv, np.float32).T).astype(np.float16)
    woT = np.ascontiguousarray(np.asarray(wo, np.float32).T).astype(np.float16)
    in_maps = []
    for b in range(B):
        in_maps.append({
            "xT": np.ascontiguousarray(x[b].T).astype(np.float16),
            "wqT": wqT, "wkT": wkT, "wvT": wvT, "woT": woT,
        })
    return in_maps


def kernel(x, wq, wk, wv, wo):
    nc = _get_nc()
    in_maps = make_in_maps(x, wq, wk, wv, wo)
    res = bass_utils.run_bass_kernel_spmd(nc, in_maps, core_ids=list(range(NCORES)))
    return np.stack([res.results[b]["y"] for b in range(B)], axis=0)
